# revision 21
# baseline (speedup 1.0000x reference)
"""Trainium2 Bass kernel for nn_MultiHeadAttention_89429809037632.

Linear attention (softplus feature map) with padding masks:
    q = query @ Wq.T ; k = key @ Wk.T ; v = key @ Wv.T   (per-head split)
    pq = softplus(q) ; pk = softplus(k) * keep(key_mask)
    kv = pk^T v (per head, plus a fused ones-column giving sum(pk))
    out = (pq @ kv) / (pq @ sum(pk)) * keep(query_mask)

Sharding across 8 NeuronCores: data-parallel over N=4 batches x
tensor-parallel over 2 head-groups (8 heads x 128 dims = 1024 output
dims each). Host transposes/packs activations and weights so the
contraction dim (D) lands on the SBUF partition axis; each core runs an
identical SPMD program on its shard, outputs are concatenated on host.

Precision plan (measured ~6e-3 end rel err vs the 2e-2 gate):
  Q and K projections run as fp8(e4m3) Double-Row matmuls (2 fp8
  weights/cell -> 256-deep contraction per pass; measured 109 ns per
  256-col matmul vs 216 ns fp16 = 1.98x). Weights are pre-scaled by 32
  on host to dodge e4m3 subnormals; the 1/32 descale rides the softplus
  EXP activation scale. The V projection stays fp16: v-path
  quantization noise does not average out in the attention sum (the
  output is ~60x smaller than v elements) and fp8 there costs 4e-2
  end-to-end error. softplus is computed directly as ln(1+e^k)
  (|k| <= ~7 so e^k is safe in fp32), 2 ACT ops per 512 cols.

Padding-mask compaction: masked keys contribute exactly zero (pk is
zeroed) and masked query rows are zeroed in the output, so the host
gathers kept rows and the device processes only ceil(kept/128)*128
tokens (~3712 of 4096 at the 10% mask rate). Padded slots carry zero
inputs and zero keep-mask, which reproduces the reference exactly.
The program is compiled for the observed kept counts (cached per
shape), so any mask still yields a correct (if slower) kernel.

Per-core program (Tile framework):
  Phase A: per 128-key chunk: K-proj via 32 DR matmuls (stationary =
    key^T d-pair, moving = packed Wk pairs), V-proj via 32 fp16
    matmuls; softplus+mask -> pk (fp16), V copied into a [v | 1]
    block layout, then 8 per-head matmuls accumulate kv_aug
    (128x129 per head) in PSUM across all key chunks.
  Phase B: per query chunk (<=512 tokens) x head: Q-proj via DR
    matmuls, softplus -> pq, one matmul per 128-query subchunk against
    kv_aug gives [num | den]; epilogue computes num * (keep/den) on
    DVE into a per-chunk staging tile shipped as one fp16 DMA.
  Matmul emission is software-pipelined (kv/num matmuls trail their
  producer chunk by one step). The first Q-projection borrows a
  phase-A PSUM tile and is emitted before the last kv matmuls, so the
  phase boundary costs almost no PE idle. Host packs kt/qt tiles so
  each chunk needs one DMA per tensor, and the weight preload is
  split/interleaved across three DGE queues so chunk 0 is not paced by
  a monolithic 8 MB transfer.
"""

import json
import os
import sys
import types

import numpy as np

for _p in ("/opt/trn_rl_repo",):
    if _p not in sys.path and os.path.isdir(_p):
        sys.path.insert(0, _p)

# ``run_bass_kernel_spmd(trace=True)`` imports antenv.axon_hooks, which not
# every image ships. Provide a stub so the import never crashes (returning
# None simply disables NTFF tracing).
try:
    import antenv.axon_hooks  # noqa: F401
except Exception:
    try:
        import antenv

        _m = types.ModuleType("antenv.axon_hooks")
        _HOOK = [None]

        def _get_hook():
            if _HOOK[0] is None:
                try:
                    from trn_agent_boot.trn_boot import _ntff_profile_via_ctypes

                    _HOOK[0] = _ntff_profile_via_ctypes("/opt/axon/libaxon_pjrt.so")
                except Exception:
                    _HOOK[0] = None
            return _HOOK[0]

        _m.get_axon_ntff_profile_hook = _get_hook
        _m.set_axon_ntff_profile_hook = lambda h: _HOOK.__setitem__(0, h)
        sys.modules["antenv.axon_hooks"] = _m
        antenv.axon_hooks = _m
    except Exception:
        pass

import ml_dtypes

import concourse.bass as bass
import concourse.bass_utils as bu
import concourse.mybir as mybir
import concourse.tile as tile

# ---------------------------------------------------------------------------
# Shim 1: this container's walrus accepts only ONE sync-wait per instruction
# ("Too many sync wait commands"); Tile attaches several. Rewrite the BIR
# JSON so excess waits ride on same-engine NoOps immediately before the
# instruction (engine streams are in-order, so this is equivalent).
# Shim 2: upload_artifacts wants a cloud bucket; keep artifacts local.
# ---------------------------------------------------------------------------
_MAX_WAITS = 1


def _split_multi_waits(bir_bytes: bytes) -> bytes:
    d = json.loads(bir_bytes)
    ctr = 0
    changed = False
    for fn in d.get("functions", []):
        for bb in fn.get("blocks", []):
            out = []
            for inst in bb.get("instructions", []):
                si = inst.get("sync_info")
                waits = (si or {}).get("on_wait") or []
                if len(waits) > _MAX_WAITS:
                    changed = True
                    idx = 0
                    while len(waits) - idx > _MAX_WAITS:
                        chunk = waits[idx : idx + _MAX_WAITS]
                        idx += _MAX_WAITS
                        ctr += 1
                        nop = {
                            "engine": inst["engine"],
                            "ins": [],
                            "outs": [],
                            "name": f"I-wsplit-{ctr}",
                            "opcode": "NoOp",
                            "sync_info": {"on_update": [], "on_wait": chunk},
                        }
                        if "debug" in inst:
                            nop["debug"] = inst["debug"]
                        out.append(nop)
                    si["on_wait"] = waits[idx:]
                out.append(inst)
            bb["instructions"] = out
    return json.dumps(d).encode() if changed else bir_bytes


if not getattr(bass.Bass, "_wait_split_shim", False):
    _orig_to_json = bass.Bass.to_json_bytes

    def _to_json_bytes(self) -> bytes:
        return _split_multi_waits(_orig_to_json(self))

    bass.Bass.to_json_bytes = _to_json_bytes
    bass.Bass._wait_split_shim = True
    bu.upload_artifacts = lambda tmpdir: tmpdir

# ---------------------------------------------------------------------------
# Problem shapes (hardcoded per contract)
# ---------------------------------------------------------------------------
N, L, D = 4, 4096, 2048  # batches, seq len (q and k), model dim
H, P = 16, 128  # heads, head dim
NCORES = 8
HL = H // 2  # heads per core (head-group of 8)
OW = HL * P  # per-core projected width (1024)
DC = D // P  # 16 contraction chunks
TP = D // 256  # 8 d-pair steps for DoubleRow
WSCALE = 32.0  # host premultiplier on fp8 weights (descaled in softplus EXP)

F32 = mybir.dt.float32
F16 = mybir.dt.float16
FP8 = mybir.dt.float8e4
DR = mybir.MatmulPerfMode.DoubleRow
E4M3 = ml_dtypes.float8_e4m3
EXP = mybir.ActivationFunctionType.Exp
LN = mybir.ActivationFunctionType.Ln
MUL = mybir.AluOpType.mult

# kv_aug per-head column offsets inside the 3-bank PSUM accumulator:
# 3 heads per 2 KiB bank (129 fp32 columns each, none crossing a bank edge).
_KV_BASE = [(h // 3) * 512 + (h % 3) * 129 for h in range(HL)]

TRACE = False  # set True (e.g. from test.py) to capture NTFF profile
LAST_EXEC_TIME_NS = None

_CACHED_NC = {}


def _build_nc(lc_a: int, jt: int) -> bass.Bass:
    """lc_a = number of 128-key chunks; jt = number of 128-query blocks."""
    from contextlib import ExitStack

    lq = jt * P  # padded query count
    lc_b = (jt + 3) // 4  # 512-query chunks (last may be partial)

    nc = bass.Bass()
    # host-packed inputs (see kernel() for the exact layouts)
    qt8p = nc.dram_tensor("qt8p", (P, jt * 2048), FP8, kind="ExternalInput")
    kt8p = nc.dram_tensor("kt8p", (P, lc_a * 2048), FP8, kind="ExternalInput")
    kt16p = nc.dram_tensor("kt16p", (P, lc_a * 2048), F16, kind="ExternalInput")
    wq8p = nc.dram_tensor("wq8p", (P, TP * 2048), FP8, kind="ExternalInput")
    wk8p = nc.dram_tensor("wk8p", (P, TP * 2048), FP8, kind="ExternalInput")
    wv = nc.dram_tensor("wv", (D, OW), F16, kind="ExternalInput")
    qm = nc.dram_tensor("qm", (P, jt), F32, kind="ExternalInput")
    km = nc.dram_tensor("km", (P, lc_a), F32, kind="ExternalInput")
    out = nc.dram_tensor("out", (lq, OW), F16, kind="ExternalOutput")

    with tile.TileContext(nc) as tc, ExitStack() as outer:
        misc = outer.enter_context(tc.tile_pool(name="misc", bufs=1))
        wqp = outer.enter_context(tc.tile_pool(name="wqe", bufs=1))
        # SBUF-side phase B pools live in the outer scope; their PSUM
        # counterparts are created only after phase A's pools close
        # (PSUM plan: A = proj 5 + kv 3; B = qp 4 + nm 4).
        qtp = outer.enter_context(tc.tile_pool(name="qt", bufs=2))
        pqp = outer.enter_context(tc.tile_pool(name="pq", bufs=3))
        stp = outer.enter_context(tc.tile_pool(name="st", bufs=3))

        qm_sb = misc.tile([P, jt], F32)
        km_sb = misc.tile([P, lc_a], F32)
        kv_sb = misc.tile([P, HL * 129], F16)
        nc.sync.dma_start(qm_sb[:], qm[:])
        nc.sync.dma_start(km_sb[:], km[:])

        # ------ Phase A: K/V projection + kv accumulation ------------------
        esA = ExitStack()
        wkvp = esA.enter_context(tc.tile_pool(name="wkv", bufs=1))
        ktp = esA.enter_context(tc.tile_pool(name="kt", bufs=3))
        pkp = esA.enter_context(tc.tile_pool(name="pk", bufs=3))
        pps = esA.enter_context(tc.tile_pool(name="projps", bufs=5, space="PSUM"))
        kvps = esA.enter_context(tc.tile_pool(name="kvps", bufs=1, space="PSUM"))
        kv_ps = kvps.tile([P, 1536], F32)

        def load_kt_chunk(c):
            t8 = ktp.tile([P, 2048], FP8, tag="kt8", name="kt8")
            t16 = ktp.tile([P, 2048], F16, tag="kt16", name="kt16")
            nc.sync.dma_start(t8[:], kt8p[:, c * 2048 : (c + 1) * 2048])
            nc.sync.dma_start(t16[:], kt16p[:, c * 2048 : (c + 1) * 2048])
            return t8, t16

        # Chunk 0-2 kt DMAs go first on the sync DGE; weights are split and
        # interleaved across the DGE queues so the opening K matmuls wait
        # on ~2.25 MB and the V matmuls see wv tiles trickling in on two
        # queues rather than queued behind a monolithic 8 MB preload.
        kt_pre = [load_kt_chunk(c) for c in range(min(3, lc_a))]

        wk_sb = wkvp.tile([P, TP * 2048], FP8, name="wk8")
        wv_sb = [
            wkvp.tile([P, OW], F16, tag=f"wv{dc}", name=f"wv{dc}")
            for dc in range(DC)
        ]
        for t in range(TP):
            # wk gates the very first matmul: split it across two DGEs
            eng = nc.gpsimd if t % 2 == 0 else nc.scalar
            eng.dma_start(
                wk_sb[:, t * 2048 : (t + 1) * 2048],
                wk8p[:, t * 2048 : (t + 1) * 2048],
            )
        for dc in range(DC):
            # all of wv rides the scalar DGE behind wk's odd half; the sync
            # DGE carries only kt chunks so the K stream never starves.
            nc.scalar.dma_start(wv_sb[dc][:], wv[dc * P : (dc + 1) * P, :])
        wq_sb = wqp.tile([P, TP * 2048], FP8, name="wq8")

        kv_bank_start = {}

        def emit_kv_mms(c, pk_sb, va_sb):
            for h in range(HL):
                bank_first = h % 3 == 0
                mm = nc.tensor.matmul(
                    kv_ps[:, _KV_BASE[h] : _KV_BASE[h] + 129],
                    pk_sb[:, h * P : (h + 1) * P],
                    va_sb[:, h * 129 : (h + 1) * 129],
                    start=(c == 0 and bank_first),
                    stop=(c == lc_a - 1),
                    skip_group_check=True,
                )
                if c == 0:
                    # start=True clears has_written for the whole PSUM bank;
                    # siblings must come after their bank's clear.
                    if bank_first:
                        kv_bank_start[h // 3] = mm
                    else:
                        tile.add_dep_helper(
                            mm.ins,
                            kv_bank_start[h // 3].ins,
                            reason="kv bank has_written clear order",
                        )

        # kv matmuls for chunk c are emitted after chunk c+1's projection
        # matmuls: their pk operand is only ready ~3us after chunk c's last
        # projection, so this keeps PE fed meanwhile.
        pending = None
        last_k_mm = None
        for c in range(lc_a):
            kt8, kt16 = kt_pre[c] if c < 3 else load_kt_chunk(c)
            if c == min(10, lc_a - 1) and last_k_mm is not None:
                # wq is only needed at the phase boundary; gating its DMA on
                # a chunk-9 matmul keeps the 2 MB transfer out of the
                # HBM-bound startup burst.
                dma = nc.gpsimd.dma_start(wq_sb[:], wq8p[:])
                tile.add_dep_helper(
                    dma.ins, last_k_mm.ins, reason="defer wq past startup"
                )
            elif c == min(10, lc_a - 1):
                nc.gpsimd.dma_start(wq_sb[:], wq8p[:])
            kp0 = pps.tile([P, 512], F32, tag="proj", name="kp0")
            kp1 = pps.tile([P, 512], F32, tag="proj", name="kp1")
            vp0 = pps.tile([P, 512], F32, tag="proj", name="vp0")
            vp1 = pps.tile([P, 512], F32, tag="proj", name="vp1")
            # K projection: fp8 DoubleRow, 256-deep contraction per step.
            # Each 512-col PSUM bank takes two 256-col matmuls; only the
            # first may carry start=True (start clears the whole bank).
            bank_start = {}
            for t in range(TP):
                stat = kt8[:, t * 256 : (t + 1) * 256].rearrange(
                    "p (i m) -> p i m", i=2
                )
                for b in range(4):
                    kp = kp0 if b < 2 else kp1
                    mov = wk_sb[:, t * 2048 : (t + 1) * 2048].rearrange(
                        "p (i o) -> p i o", i=2
                    )[:, :, b * 256 : (b + 1) * 256]
                    first = b % 2 == 0
                    mm = nc.tensor.matmul(
                        kp[:, (b % 2) * 256 : (b % 2 + 1) * 256],
                        stat,
                        mov,
                        start=(t == 0 and first),
                        stop=(t == TP - 1),
                        perf_mode=DR,
                        skip_group_check=True,
                    )
                    if t == 0:
                        if first:
                            bank_start[b // 2] = mm
                        else:
                            tile.add_dep_helper(
                                mm.ins,
                                bank_start[b // 2].ins,
                                reason="kp bank has_written clear order",
                            )
                    last_k_mm = mm
            # V projection: fp16 full-rate (fp8 noise here does not average
            # out in the attention sum; costs 4e-2 end-to-end error).
            for dc in range(DC):
                lhsT = kt16[:, dc * P : (dc + 1) * P]
                st = dict(start=(dc == 0), stop=(dc == DC - 1))
                nc.tensor.matmul(vp0[:], lhsT, wv_sb[dc][:, 0:512], **st)
                nc.tensor.matmul(vp1[:], lhsT, wv_sb[dc][:, 512:1024], **st)

            if pending is not None:
                emit_kv_mms(*pending)

            # softplus(k) = ln(1 + e^k); kp holds 32k so EXP descales.
            pk_sb = pkp.tile([P, OW], F16, tag="pk", name="pk")
            for half, kp in ((0, kp0), (1, kp1)):
                sa = pkp.tile([P, 512], F32, tag=f"sa{half}", name=f"sa{half}")
                nc.scalar.activation(sa[:], kp[:], EXP, scale=1.0 / WSCALE)
                nc.scalar.activation(
                    pk_sb[:, half * 512 : (half + 1) * 512], sa[:], LN, bias=1.0
                )
            nc.vector.tensor_scalar_mul(pk_sb[:], pk_sb[:], km_sb[:, c : c + 1])

            va_sb = pkp.tile([P, HL * 129], F16, tag="vaug", name="va")
            va3 = va_sb[:].rearrange("p (h x) -> p h x", x=129)
            nc.vector.memset(va3[:, :, 128:129], 1.0)
            nc.vector.tensor_copy(
                va3[:, 0:4, 0:P], vp0[:].rearrange("p (h x) -> p h x", x=P)
            )
            nc.vector.tensor_copy(
                va3[:, 4:8, 0:P], vp1[:].rearrange("p (h x) -> p h x", x=P)
            )
            pending = (c, pk_sb, va_sb)

        # ------ Phase B: Q projection + attention epilogue -----------------
        st_tiles = {}
        nmps = None
        scp = pqp  # sc tiles ride the pq pool

        def emit_num(lc, h, jcnt, pq_sb):
            # results stage into st (partition=l%128, cols j*OW+o); heads
            # 0-3 ship at h==3 so only half the staging waits on the tail.
            if h == 0:
                st_tiles[lc] = stp.tile([P, 4 * OW], F16, tag="st", name="st")
            st = st_tiles[lc]
            for j in range(jcnt):
                nm = nmps.tile([P, 129], F32, tag="nm", name="nm")
                nc.tensor.matmul(
                    nm[:],
                    pq_sb[:, j * P : (j + 1) * P],
                    kv_sb[:, h * 129 : h * 129 + 129],
                    start=True,
                    stop=True,
                )
                sc = scp.tile([P, 1], F32, tag="sc", name="sc")
                nc.vector.reciprocal(sc[:], nm[:, 128:129])
                col = lc * 4 + j
                nc.vector.tensor_scalar(
                    st[:, j * OW + h * P : j * OW + (h + 1) * P],
                    nm[:, 0:P],
                    sc[:, 0:1],
                    qm_sb[:, col : col + 1],
                    MUL,
                    MUL,
                )
            if h == 3 or h == HL - 1:
                half = slice(0, 512) if h == 3 else slice(512, 1024)
                nc.sync.dma_start(
                    out[lc * 512 : lc * 512 + jcnt * P, half].rearrange(
                        "(j p) o -> p j o", p=P
                    ),
                    st[:, 0 : jcnt * OW]
                    .rearrange("p (j o) -> p j o", o=OW)[:, :, half],
                )
                if h == HL - 1:
                    del st_tiles[lc]

        # num matmuls for step (lc,h) are emitted after step (lc,h)+1's
        # projection matmuls (pq is ~1.5us of ACT behind qp). The first
        # Q-projection borrows a phase-A PSUM tile and runs BEFORE the
        # last kv matmuls, hiding the final chunk's softplus drain.
        pendingB = None
        qpps = None
        for lc in range(lc_b):
            jcnt = min(4, jt - lc * 4)
            qt8 = qtp.tile([P, 2048 * jcnt], FP8, tag="qt8", name="qt8")
            nc.gpsimd.dma_start(
                qt8[:], qt8p[:, lc * 8192 : lc * 8192 + 2048 * jcnt]
            )
            nt = jcnt * P  # tokens this chunk
            for h in range(HL):
                first_step = lc == 0 and h == 0
                if first_step:
                    qp = pps.tile([P, 512], F32, tag="proj", name="qp0")
                else:
                    qp = qpps.tile([P, 512], F32, tag="qp", name="qp")
                qp_start = None
                for t in range(TP):
                    stat = wq_sb[
                        :, t * 2048 + h * 256 : t * 2048 + (h + 1) * 256
                    ].rearrange("p (i o) -> p i o", i=2)
                    for b0 in range(0, nt, 256):
                        bn = min(256, nt - b0)
                        mov = qt8[
                            :, t * nt * 2 : (t + 1) * nt * 2
                        ].rearrange("p (i n) -> p i n", i=2)[
                            :, :, b0 : b0 + bn
                        ]
                        mm = nc.tensor.matmul(
                            qp[:, b0 : b0 + bn],
                            stat,
                            mov,
                            start=(t == 0 and b0 == 0),
                            stop=(t == TP - 1),
                            perf_mode=DR,
                            skip_group_check=True,
                        )
                        if t == 0:
                            if b0 == 0:
                                qp_start = mm
                            else:
                                tile.add_dep_helper(
                                    mm.ins,
                                    qp_start.ins,
                                    reason="qp bank has_written clear order",
                                )
                if pendingB is not None:
                    emit_num(*pendingB)
                pq_sb = pqp.tile([P, 512], F16, tag="pq", name="pq")
                sa = pqp.tile([P, 512], F32, tag="sqa", name="sqa")
                nc.scalar.activation(
                    sa[:, 0:nt], qp[:, 0:nt], EXP, scale=1.0 / WSCALE
                )
                nc.scalar.activation(pq_sb[:, 0:nt], sa[:, 0:nt], LN, bias=1.0)
                pendingB = (lc, h, jcnt, pq_sb)
                if first_step:
                    # phase A epilogue rides behind the first Q-projection
                    emit_kv_mms(*pending)
                    for hh in range(HL):
                        nc.vector.tensor_copy(
                            kv_sb[:, hh * 129 : (hh + 1) * 129],
                            kv_ps[:, _KV_BASE[hh] : _KV_BASE[hh] + 129],
                        )
                    esA.close()
                    qpps = outer.enter_context(
                        tc.tile_pool(name="qpps", bufs=4, space="PSUM")
                    )
                    nmps = outer.enter_context(
                        tc.tile_pool(name="nmps", bufs=4, space="PSUM")
                    )
        emit_num(*pendingB)
    return nc


def _get_nc(lc_a: int, jt: int) -> bass.Bass:
    if (lc_a, jt) not in _CACHED_NC:
        _CACHED_NC[(lc_a, jt)] = _build_nc(lc_a, jt)
    return _CACHED_NC[(lc_a, jt)]


def kernel(query, key, Wq, Wk, Wv, query_padding_mask, key_padding_mask):
    global LAST_EXEC_TIME_NS
    query = np.asarray(query, dtype=np.float32)
    key = np.asarray(key, dtype=np.float32)
    Wq = np.asarray(Wq, dtype=np.float32)
    Wk = np.asarray(Wk, dtype=np.float32)
    Wv = np.asarray(Wv, dtype=np.float32)
    qmask = np.asarray(query_padding_mask)
    kmask = np.asarray(key_padding_mask)

    # Compact away masked tokens (exact: masked keys contribute zero via
    # the zeroed keep-mask; masked query rows are zeroed on scatter-back).
    kept_k = [np.flatnonzero(~kmask[n]) for n in range(N)]
    kept_q = [np.flatnonzero(~qmask[n]) for n in range(N)]
    lc_a = max(1, -(-max(len(i) for i in kept_k) // P))
    jt = max(1, -(-max(len(i) for i in kept_q) // P))
    lk, lq = lc_a * P, jt * P

    nc = _get_nc(lc_a, jt)

    # Packed layouts (p is always the SBUF partition index, d = 256t+128i+p):
    #   qt8p[p, lc*8192 + t*(2*nt) + i*nt + n] = fp8(query_c[lc*512+n, d])
    #   kt8p[p, c*2048 + t*256 + i*128 + m]    = fp8(key_c[c*128+m, d])
    #   kt16p[p, c*2048 + dc*128 + m]          = fp16(key_c[c*128+m, dc*128+p])
    #   wk8p[p, t*2048 + i*1024 + o]           = fp8(32*Wk[g*OW+o, d])
    #   wq8p[p, t*2048 + h*256 + i*128 + o]    = fp8(32*Wq[g*OW+h*128+o, d])
    per_n = {}
    for n in range(N):
        kc = np.zeros((lk, D), np.float32)
        kc[: len(kept_k[n])] = key[n][kept_k[n]]
        qc = np.zeros((lq, D), np.float32)
        qc[: len(kept_q[n])] = query[n][kept_q[n]]
        kmk = np.zeros(lk, np.float32)
        kmk[: len(kept_k[n])] = 1.0
        qmk = np.zeros(lq, np.float32)
        qmk[: len(kept_q[n])] = 1.0
        q8 = qc.astype(E4M3)
        k8 = kc.astype(E4M3)
        k16 = kc.astype(np.float16)
        # qt8p: per 512-token chunk (last may be short), layout t-major
        qt_parts = []
        for lc in range((jt + 3) // 4):
            nt = min(512, lq - lc * 512)
            blk = q8[lc * 512 : lc * 512 + nt]  # (nt, D)
            qt_parts.append(
                blk.reshape(nt, TP, 2, P).transpose(3, 1, 2, 0).reshape(P, -1)
            )
        per_n[n] = {
            "qt8p": np.ascontiguousarray(np.concatenate(qt_parts, axis=1)),
            "kt8p": np.ascontiguousarray(
                k8.reshape(lc_a, P, TP, 2, P)
                .transpose(4, 0, 2, 3, 1)
                .reshape(P, -1)
            ),
            "kt16p": np.ascontiguousarray(
                k16.reshape(lc_a, P, DC, P).transpose(3, 0, 2, 1).reshape(P, -1)
            ),
            "qm": np.ascontiguousarray(qmk.reshape(jt, P).T),
            "km": np.ascontiguousarray(kmk.reshape(lc_a, P).T),
        }
    per_g = {}
    for g in range(2):
        sl = slice(g * OW, (g + 1) * OW)
        wq8 = (Wq[sl, :].T * WSCALE).astype(E4M3)  # (D, OW)
        wk8 = (Wk[sl, :].T * WSCALE).astype(E4M3)
        per_g[g] = {
            "wq8p": np.ascontiguousarray(
                wq8.reshape(TP, 2, P, HL, P).transpose(2, 0, 3, 1, 4).reshape(P, -1)
            ),
            "wk8p": np.ascontiguousarray(
                wk8.reshape(TP, 2, P, OW).transpose(2, 0, 1, 3).reshape(P, -1)
            ),
            "wv": np.ascontiguousarray(Wv[sl, :].T.astype(np.float16)),
        }

    in_maps = []
    for c in range(NCORES):
        n, g = c // 2, c % 2
        in_maps.append({**per_n[n], **per_g[g]})

    res = bu.run_bass_kernel_spmd(
        nc, in_maps, core_ids=list(range(NCORES)), trace=TRACE
    )
    LAST_EXEC_TIME_NS = res.exec_time_ns

    full = np.zeros((N, L, D), dtype=np.float32)
    for c in range(NCORES):
        n, g = c // 2, c % 2
        o = res.results[c]["out"].astype(np.float32)
        full[n, kept_q[n], g * OW : (g + 1) * OW] = o[: len(kept_q[n])]
    return full


# revision 22
# speedup vs baseline: 1.0022x; 1.0022x over previous
"""Trainium2 Bass kernel for nn_MultiHeadAttention_89429809037632.

Linear attention (softplus feature map) with padding masks:
    q = query @ Wq.T ; k = key @ Wk.T ; v = key @ Wv.T   (per-head split)
    pq = softplus(q) ; pk = softplus(k) * keep(key_mask)
    kv = pk^T v (per head, plus a fused ones-column giving sum(pk))
    out = (pq @ kv) / (pq @ sum(pk)) * keep(query_mask)

Sharding across 8 NeuronCores: data-parallel over N=4 batches x
tensor-parallel over 2 head-groups (8 heads x 128 dims = 1024 output
dims each). Host transposes/packs activations and weights so the
contraction dim (D) lands on the SBUF partition axis; each core runs an
identical SPMD program on its shard, outputs are concatenated on host.

Precision plan (measured ~6e-3 end rel err vs the 2e-2 gate):
  Q and K projections run as fp8(e4m3) Double-Row matmuls (2 fp8
  weights/cell -> 256-deep contraction per pass; measured 109 ns per
  256-col matmul vs 216 ns fp16 = 1.98x). Weights are pre-scaled by 32
  on host to dodge e4m3 subnormals; the 1/32 descale rides the softplus
  EXP activation scale. The V projection stays fp16: v-path
  quantization noise does not average out in the attention sum (the
  output is ~60x smaller than v elements) and fp8 there costs 4e-2
  end-to-end error. softplus is computed directly as ln(1+e^k)
  (|k| <= ~7 so e^k is safe in fp32), 2 ACT ops per 512 cols.

Padding-mask compaction: masked keys contribute exactly zero (pk is
zeroed) and masked query rows are zeroed in the output, so the host
gathers kept rows and the device processes only ceil(kept/128)*128
tokens (~3712 of 4096 at the 10% mask rate). Padded slots carry zero
inputs and zero keep-mask, which reproduces the reference exactly.
The program is compiled for the observed kept counts (cached per
shape), so any mask still yields a correct (if slower) kernel.

Per-core program (Tile framework):
  Phase A: per 128-key chunk: K-proj via 32 DR matmuls (stationary =
    key^T d-pair, moving = packed Wk pairs), V-proj via 32 fp16
    matmuls; softplus+mask -> pk (fp16), V copied into a [v | 1]
    block layout, then 8 per-head matmuls accumulate kv_aug
    (128x129 per head) in PSUM across all key chunks.
  Phase B: per query chunk (<=512 tokens) x head: Q-proj via DR
    matmuls, softplus -> pq, one matmul per 128-query subchunk against
    kv_aug gives [num | den]; epilogue computes num * (keep/den) on
    DVE into a per-chunk staging tile shipped as one fp16 DMA.
  Matmul emission is software-pipelined (kv/num matmuls trail their
  producer chunk by one step). The first Q-projection borrows a
  phase-A PSUM tile and is emitted before the last kv matmuls, so the
  phase boundary costs almost no PE idle. Host packs kt/qt tiles so
  each chunk needs one DMA per tensor, and the weight preload is
  split/interleaved across three DGE queues so chunk 0 is not paced by
  a monolithic 8 MB transfer.
"""

import json
import os
import sys
import types

import numpy as np

for _p in ("/opt/trn_rl_repo",):
    if _p not in sys.path and os.path.isdir(_p):
        sys.path.insert(0, _p)

# ``run_bass_kernel_spmd(trace=True)`` imports antenv.axon_hooks, which not
# every image ships. Provide a stub so the import never crashes (returning
# None simply disables NTFF tracing).
try:
    import antenv.axon_hooks  # noqa: F401
except Exception:
    try:
        import antenv

        _m = types.ModuleType("antenv.axon_hooks")
        _HOOK = [None]

        def _get_hook():
            if _HOOK[0] is None:
                try:
                    from trn_agent_boot.trn_boot import _ntff_profile_via_ctypes

                    _HOOK[0] = _ntff_profile_via_ctypes("/opt/axon/libaxon_pjrt.so")
                except Exception:
                    _HOOK[0] = None
            return _HOOK[0]

        _m.get_axon_ntff_profile_hook = _get_hook
        _m.set_axon_ntff_profile_hook = lambda h: _HOOK.__setitem__(0, h)
        sys.modules["antenv.axon_hooks"] = _m
        antenv.axon_hooks = _m
    except Exception:
        pass

import ml_dtypes

import concourse.bass as bass
import concourse.bass_utils as bu
import concourse.mybir as mybir
import concourse.tile as tile

# ---------------------------------------------------------------------------
# Shim 1: this container's walrus accepts only ONE sync-wait per instruction
# ("Too many sync wait commands"); Tile attaches several. Rewrite the BIR
# JSON so excess waits ride on same-engine NoOps immediately before the
# instruction (engine streams are in-order, so this is equivalent).
# Shim 2: upload_artifacts wants a cloud bucket; keep artifacts local.
# ---------------------------------------------------------------------------
_MAX_WAITS = 1


def _split_multi_waits(bir_bytes: bytes) -> bytes:
    d = json.loads(bir_bytes)
    ctr = 0
    changed = False
    for fn in d.get("functions", []):
        for bb in fn.get("blocks", []):
            out = []
            for inst in bb.get("instructions", []):
                si = inst.get("sync_info")
                waits = (si or {}).get("on_wait") or []
                if len(waits) > _MAX_WAITS:
                    changed = True
                    idx = 0
                    while len(waits) - idx > _MAX_WAITS:
                        chunk = waits[idx : idx + _MAX_WAITS]
                        idx += _MAX_WAITS
                        ctr += 1
                        nop = {
                            "engine": inst["engine"],
                            "ins": [],
                            "outs": [],
                            "name": f"I-wsplit-{ctr}",
                            "opcode": "NoOp",
                            "sync_info": {"on_update": [], "on_wait": chunk},
                        }
                        if "debug" in inst:
                            nop["debug"] = inst["debug"]
                        out.append(nop)
                    si["on_wait"] = waits[idx:]
                out.append(inst)
            bb["instructions"] = out
    return json.dumps(d).encode() if changed else bir_bytes


if not getattr(bass.Bass, "_wait_split_shim", False):
    _orig_to_json = bass.Bass.to_json_bytes

    def _to_json_bytes(self) -> bytes:
        return _split_multi_waits(_orig_to_json(self))

    bass.Bass.to_json_bytes = _to_json_bytes
    bass.Bass._wait_split_shim = True
    bu.upload_artifacts = lambda tmpdir: tmpdir

# ---------------------------------------------------------------------------
# Problem shapes (hardcoded per contract)
# ---------------------------------------------------------------------------
N, L, D = 4, 4096, 2048  # batches, seq len (q and k), model dim
H, P = 16, 128  # heads, head dim
NCORES = 8
HL = H // 2  # heads per core (head-group of 8)
OW = HL * P  # per-core projected width (1024)
DC = D // P  # 16 contraction chunks
TP = D // 256  # 8 d-pair steps for DoubleRow
WSCALE = 32.0  # host premultiplier on fp8 weights (descaled in softplus EXP)

F32 = mybir.dt.float32
F16 = mybir.dt.float16
FP8 = mybir.dt.float8e4
DR = mybir.MatmulPerfMode.DoubleRow
E4M3 = ml_dtypes.float8_e4m3
EXP = mybir.ActivationFunctionType.Exp
LN = mybir.ActivationFunctionType.Ln
MUL = mybir.AluOpType.mult

# kv_aug per-head column offsets inside the 3-bank PSUM accumulator:
# 3 heads per 2 KiB bank (129 fp32 columns each, none crossing a bank edge).
_KV_BASE = [(h // 3) * 512 + (h % 3) * 129 for h in range(HL)]

TRACE = False  # set True (e.g. from test.py) to capture NTFF profile
LAST_EXEC_TIME_NS = None

_CACHED_NC = {}


def _build_nc(lc_a: int, jt: int) -> bass.Bass:
    """lc_a = number of 128-key chunks; jt = number of 128-query blocks."""
    from contextlib import ExitStack

    lq = jt * P  # padded query count
    lc_b = (jt + 3) // 4  # 512-query chunks (last may be partial)

    nc = bass.Bass()
    # host-packed inputs (see kernel() for the exact layouts)
    qt8p = nc.dram_tensor("qt8p", (P, jt * 2048), FP8, kind="ExternalInput")
    kt8p = nc.dram_tensor("kt8p", (P, lc_a * 2048), FP8, kind="ExternalInput")
    kt16p = nc.dram_tensor("kt16p", (P, lc_a * 2048), F16, kind="ExternalInput")
    wq8p = nc.dram_tensor("wq8p", (P, TP * 2048), FP8, kind="ExternalInput")
    wk8p = nc.dram_tensor("wk8p", (P, TP * 2048), FP8, kind="ExternalInput")
    wv = nc.dram_tensor("wv", (D, OW), F16, kind="ExternalInput")
    qm = nc.dram_tensor("qm", (P, jt), F32, kind="ExternalInput")
    km = nc.dram_tensor("km", (P, lc_a), F32, kind="ExternalInput")
    out = nc.dram_tensor("out", (lq, OW), F16, kind="ExternalOutput")

    with tile.TileContext(nc) as tc, ExitStack() as outer:
        misc = outer.enter_context(tc.tile_pool(name="misc", bufs=1))
        wqp = outer.enter_context(tc.tile_pool(name="wqe", bufs=1))
        # SBUF-side phase B pools live in the outer scope; their PSUM
        # counterparts are created only after phase A's pools close
        # (PSUM plan: A = proj 5 + kv 3; B = qp 4 + nm 4).
        qtp = outer.enter_context(tc.tile_pool(name="qt", bufs=2))
        pqp = outer.enter_context(tc.tile_pool(name="pq", bufs=3))
        stp = outer.enter_context(tc.tile_pool(name="st", bufs=3))

        qm_sb = misc.tile([P, jt], F32)
        km_sb = misc.tile([P, lc_a], F32)
        kv_sb = misc.tile([P, HL * 129], F16)
        nc.sync.dma_start(qm_sb[:], qm[:])
        nc.sync.dma_start(km_sb[:], km[:])

        # ------ Phase A: K/V projection + kv accumulation ------------------
        esA = ExitStack()
        wkvp = esA.enter_context(tc.tile_pool(name="wkv", bufs=1))
        ktp = esA.enter_context(tc.tile_pool(name="kt", bufs=3))
        pkp = esA.enter_context(tc.tile_pool(name="pk", bufs=3))
        pps = esA.enter_context(tc.tile_pool(name="projps", bufs=5, space="PSUM"))
        kvps = esA.enter_context(tc.tile_pool(name="kvps", bufs=1, space="PSUM"))
        kv_ps = kvps.tile([P, 1536], F32)

        def load_kt_chunk(c):
            t8 = ktp.tile([P, 2048], FP8, tag="kt8", name="kt8")
            t16 = ktp.tile([P, 2048], F16, tag="kt16", name="kt16")
            nc.sync.dma_start(t8[:], kt8p[:, c * 2048 : (c + 1) * 2048])
            nc.sync.dma_start(t16[:], kt16p[:, c * 2048 : (c + 1) * 2048])
            return t8, t16

        # Chunk 0-2 kt DMAs go first on the sync DGE; weights are split and
        # interleaved across the DGE queues so the opening K matmuls wait
        # on ~2.25 MB and the V matmuls see wv tiles trickling in on two
        # queues rather than queued behind a monolithic 8 MB preload.
        kt_pre = [load_kt_chunk(c) for c in range(min(3, lc_a))]

        wk_sb = wkvp.tile([P, TP * 2048], FP8, name="wk8")
        wv_sb = [
            wkvp.tile([P, OW], F16, tag=f"wv{dc}", name=f"wv{dc}")
            for dc in range(DC)
        ]
        for t in range(TP):
            # wk gates the very first matmul: split it across two DGEs
            eng = nc.gpsimd if t % 2 == 0 else nc.scalar
            eng.dma_start(
                wk_sb[:, t * 2048 : (t + 1) * 2048],
                wk8p[:, t * 2048 : (t + 1) * 2048],
            )
        for dc in range(DC):
            # wv splits across the scalar DGE (behind wk's odd half) and the
            # sync DGE (behind the kt prefetch); with wq gated out of the
            # startup burst both halves land before the first V matmuls.
            eng = nc.scalar if dc % 2 == 0 else nc.sync
            eng.dma_start(wv_sb[dc][:], wv[dc * P : (dc + 1) * P, :])
        wq_sb = wqp.tile([P, TP * 2048], FP8, name="wq8")

        kv_bank_start = {}

        def emit_kv_mms(c, pk_sb, va_sb):
            for h in range(HL):
                bank_first = h % 3 == 0
                mm = nc.tensor.matmul(
                    kv_ps[:, _KV_BASE[h] : _KV_BASE[h] + 129],
                    pk_sb[:, h * P : (h + 1) * P],
                    va_sb[:, h * 129 : (h + 1) * 129],
                    start=(c == 0 and bank_first),
                    stop=(c == lc_a - 1),
                    skip_group_check=True,
                )
                if c == 0:
                    # start=True clears has_written for the whole PSUM bank;
                    # siblings must come after their bank's clear.
                    if bank_first:
                        kv_bank_start[h // 3] = mm
                    else:
                        tile.add_dep_helper(
                            mm.ins,
                            kv_bank_start[h // 3].ins,
                            reason="kv bank has_written clear order",
                        )

        # kv matmuls for chunk c are emitted after chunk c+1's projection
        # matmuls: their pk operand is only ready ~3us after chunk c's last
        # projection, so this keeps PE fed meanwhile.
        pending = None
        last_k_mm = None
        for c in range(lc_a):
            kt8, kt16 = kt_pre[c] if c < 3 else load_kt_chunk(c)
            if c == min(10, lc_a - 1) and last_k_mm is not None:
                # wq is only needed at the phase boundary; gating its DMA on
                # a chunk-9 matmul keeps the 2 MB transfer out of the
                # HBM-bound startup burst.
                dma = nc.gpsimd.dma_start(wq_sb[:], wq8p[:])
                tile.add_dep_helper(
                    dma.ins, last_k_mm.ins, reason="defer wq past startup"
                )
            elif c == min(10, lc_a - 1):
                nc.gpsimd.dma_start(wq_sb[:], wq8p[:])
            kp0 = pps.tile([P, 512], F32, tag="proj", name="kp0")
            kp1 = pps.tile([P, 512], F32, tag="proj", name="kp1")
            vp0 = pps.tile([P, 512], F32, tag="proj", name="vp0")
            vp1 = pps.tile([P, 512], F32, tag="proj", name="vp1")
            # K projection: fp8 DoubleRow, 256-deep contraction per step.
            # Each 512-col PSUM bank takes two 256-col matmuls; only the
            # first may carry start=True (start clears the whole bank).
            bank_start = {}
            for t in range(TP):
                stat = kt8[:, t * 256 : (t + 1) * 256].rearrange(
                    "p (i m) -> p i m", i=2
                )
                for b in range(4):
                    kp = kp0 if b < 2 else kp1
                    mov = wk_sb[:, t * 2048 : (t + 1) * 2048].rearrange(
                        "p (i o) -> p i o", i=2
                    )[:, :, b * 256 : (b + 1) * 256]
                    first = b % 2 == 0
                    mm = nc.tensor.matmul(
                        kp[:, (b % 2) * 256 : (b % 2 + 1) * 256],
                        stat,
                        mov,
                        start=(t == 0 and first),
                        stop=(t == TP - 1),
                        perf_mode=DR,
                        skip_group_check=True,
                    )
                    if t == 0:
                        if first:
                            bank_start[b // 2] = mm
                        else:
                            tile.add_dep_helper(
                                mm.ins,
                                bank_start[b // 2].ins,
                                reason="kp bank has_written clear order",
                            )
                    last_k_mm = mm
            # V projection: fp16 full-rate (fp8 noise here does not average
            # out in the attention sum; costs 4e-2 end-to-end error).
            for dc in range(DC):
                lhsT = kt16[:, dc * P : (dc + 1) * P]
                st = dict(start=(dc == 0), stop=(dc == DC - 1))
                nc.tensor.matmul(vp0[:], lhsT, wv_sb[dc][:, 0:512], **st)
                nc.tensor.matmul(vp1[:], lhsT, wv_sb[dc][:, 512:1024], **st)

            if pending is not None:
                emit_kv_mms(*pending)

            # softplus(k) = ln(1 + e^k); kp holds 32k so EXP descales.
            pk_sb = pkp.tile([P, OW], F16, tag="pk", name="pk")
            for half, kp in ((0, kp0), (1, kp1)):
                sa = pkp.tile([P, 512], F32, tag=f"sa{half}", name=f"sa{half}")
                nc.scalar.activation(sa[:], kp[:], EXP, scale=1.0 / WSCALE)
                nc.scalar.activation(
                    pk_sb[:, half * 512 : (half + 1) * 512], sa[:], LN, bias=1.0
                )
            nc.vector.tensor_scalar_mul(pk_sb[:], pk_sb[:], km_sb[:, c : c + 1])

            va_sb = pkp.tile([P, HL * 129], F16, tag="vaug", name="va")
            va3 = va_sb[:].rearrange("p (h x) -> p h x", x=129)
            nc.vector.memset(va3[:, :, 128:129], 1.0)
            nc.vector.tensor_copy(
                va3[:, 0:4, 0:P], vp0[:].rearrange("p (h x) -> p h x", x=P)
            )
            nc.vector.tensor_copy(
                va3[:, 4:8, 0:P], vp1[:].rearrange("p (h x) -> p h x", x=P)
            )
            pending = (c, pk_sb, va_sb)

        # ------ Phase B: Q projection + attention epilogue -----------------
        st_tiles = {}
        nmps = None
        scp = pqp  # sc tiles ride the pq pool

        def emit_num(lc, h, jcnt, pq_sb):
            # results stage into st (partition=l%128, cols j*OW+o); heads
            # 0-3 ship at h==3 so only half the staging waits on the tail.
            if h == 0:
                st_tiles[lc] = stp.tile([P, 4 * OW], F16, tag="st", name="st")
            st = st_tiles[lc]
            for j in range(jcnt):
                nm = nmps.tile([P, 129], F32, tag="nm", name="nm")
                nc.tensor.matmul(
                    nm[:],
                    pq_sb[:, j * P : (j + 1) * P],
                    kv_sb[:, h * 129 : h * 129 + 129],
                    start=True,
                    stop=True,
                )
                sc = scp.tile([P, 1], F32, tag="sc", name="sc")
                nc.vector.reciprocal(sc[:], nm[:, 128:129])
                col = lc * 4 + j
                nc.vector.tensor_scalar(
                    st[:, j * OW + h * P : j * OW + (h + 1) * P],
                    nm[:, 0:P],
                    sc[:, 0:1],
                    qm_sb[:, col : col + 1],
                    MUL,
                    MUL,
                )
            if h == 3 or h == HL - 1:
                half = slice(0, 512) if h == 3 else slice(512, 1024)
                nc.sync.dma_start(
                    out[lc * 512 : lc * 512 + jcnt * P, half].rearrange(
                        "(j p) o -> p j o", p=P
                    ),
                    st[:, 0 : jcnt * OW]
                    .rearrange("p (j o) -> p j o", o=OW)[:, :, half],
                )
                if h == HL - 1:
                    del st_tiles[lc]

        # num matmuls for step (lc,h) are emitted after step (lc,h)+1's
        # projection matmuls (pq is ~1.5us of ACT behind qp). The first
        # Q-projection borrows a phase-A PSUM tile and runs BEFORE the
        # last kv matmuls, hiding the final chunk's softplus drain.
        pendingB = None
        qpps = None
        for lc in range(lc_b):
            jcnt = min(4, jt - lc * 4)
            qt8 = qtp.tile([P, 2048 * jcnt], FP8, tag="qt8", name="qt8")
            nc.gpsimd.dma_start(
                qt8[:], qt8p[:, lc * 8192 : lc * 8192 + 2048 * jcnt]
            )
            nt = jcnt * P  # tokens this chunk
            for h in range(HL):
                first_step = lc == 0 and h == 0
                if first_step:
                    qp = pps.tile([P, 512], F32, tag="proj", name="qp0")
                else:
                    qp = qpps.tile([P, 512], F32, tag="qp", name="qp")
                qp_start = None
                for t in range(TP):
                    stat = wq_sb[
                        :, t * 2048 + h * 256 : t * 2048 + (h + 1) * 256
                    ].rearrange("p (i o) -> p i o", i=2)
                    for b0 in range(0, nt, 256):
                        bn = min(256, nt - b0)
                        mov = qt8[
                            :, t * nt * 2 : (t + 1) * nt * 2
                        ].rearrange("p (i n) -> p i n", i=2)[
                            :, :, b0 : b0 + bn
                        ]
                        mm = nc.tensor.matmul(
                            qp[:, b0 : b0 + bn],
                            stat,
                            mov,
                            start=(t == 0 and b0 == 0),
                            stop=(t == TP - 1),
                            perf_mode=DR,
                            skip_group_check=True,
                        )
                        if t == 0:
                            if b0 == 0:
                                qp_start = mm
                            else:
                                tile.add_dep_helper(
                                    mm.ins,
                                    qp_start.ins,
                                    reason="qp bank has_written clear order",
                                )
                if pendingB is not None:
                    emit_num(*pendingB)
                pq_sb = pqp.tile([P, 512], F16, tag="pq", name="pq")
                sa = pqp.tile([P, 512], F32, tag="sqa", name="sqa")
                nc.scalar.activation(
                    sa[:, 0:nt], qp[:, 0:nt], EXP, scale=1.0 / WSCALE
                )
                nc.scalar.activation(pq_sb[:, 0:nt], sa[:, 0:nt], LN, bias=1.0)
                pendingB = (lc, h, jcnt, pq_sb)
                if first_step:
                    # phase A epilogue rides behind the first Q-projection
                    emit_kv_mms(*pending)
                    for hh in range(HL):
                        nc.vector.tensor_copy(
                            kv_sb[:, hh * 129 : (hh + 1) * 129],
                            kv_ps[:, _KV_BASE[hh] : _KV_BASE[hh] + 129],
                        )
                    esA.close()
                    qpps = outer.enter_context(
                        tc.tile_pool(name="qpps", bufs=4, space="PSUM")
                    )
                    nmps = outer.enter_context(
                        tc.tile_pool(name="nmps", bufs=4, space="PSUM")
                    )
        emit_num(*pendingB)
    return nc


def _get_nc(lc_a: int, jt: int) -> bass.Bass:
    if (lc_a, jt) not in _CACHED_NC:
        _CACHED_NC[(lc_a, jt)] = _build_nc(lc_a, jt)
    return _CACHED_NC[(lc_a, jt)]


def kernel(query, key, Wq, Wk, Wv, query_padding_mask, key_padding_mask):
    global LAST_EXEC_TIME_NS
    query = np.asarray(query, dtype=np.float32)
    key = np.asarray(key, dtype=np.float32)
    Wq = np.asarray(Wq, dtype=np.float32)
    Wk = np.asarray(Wk, dtype=np.float32)
    Wv = np.asarray(Wv, dtype=np.float32)
    qmask = np.asarray(query_padding_mask)
    kmask = np.asarray(key_padding_mask)

    # Compact away masked tokens (exact: masked keys contribute zero via
    # the zeroed keep-mask; masked query rows are zeroed on scatter-back).
    kept_k = [np.flatnonzero(~kmask[n]) for n in range(N)]
    kept_q = [np.flatnonzero(~qmask[n]) for n in range(N)]
    lc_a = max(1, -(-max(len(i) for i in kept_k) // P))
    jt = max(1, -(-max(len(i) for i in kept_q) // P))
    lk, lq = lc_a * P, jt * P

    nc = _get_nc(lc_a, jt)

    # Packed layouts (p is always the SBUF partition index, d = 256t+128i+p):
    #   qt8p[p, lc*8192 + t*(2*nt) + i*nt + n] = fp8(query_c[lc*512+n, d])
    #   kt8p[p, c*2048 + t*256 + i*128 + m]    = fp8(key_c[c*128+m, d])
    #   kt16p[p, c*2048 + dc*128 + m]          = fp16(key_c[c*128+m, dc*128+p])
    #   wk8p[p, t*2048 + i*1024 + o]           = fp8(32*Wk[g*OW+o, d])
    #   wq8p[p, t*2048 + h*256 + i*128 + o]    = fp8(32*Wq[g*OW+h*128+o, d])
    per_n = {}
    for n in range(N):
        kc = np.zeros((lk, D), np.float32)
        kc[: len(kept_k[n])] = key[n][kept_k[n]]
        qc = np.zeros((lq, D), np.float32)
        qc[: len(kept_q[n])] = query[n][kept_q[n]]
        kmk = np.zeros(lk, np.float32)
        kmk[: len(kept_k[n])] = 1.0
        qmk = np.zeros(lq, np.float32)
        qmk[: len(kept_q[n])] = 1.0
        q8 = qc.astype(E4M3)
        k8 = kc.astype(E4M3)
        k16 = kc.astype(np.float16)
        # qt8p: per 512-token chunk (last may be short), layout t-major
        qt_parts = []
        for lc in range((jt + 3) // 4):
            nt = min(512, lq - lc * 512)
            blk = q8[lc * 512 : lc * 512 + nt]  # (nt, D)
            qt_parts.append(
                blk.reshape(nt, TP, 2, P).transpose(3, 1, 2, 0).reshape(P, -1)
            )
        per_n[n] = {
            "qt8p": np.ascontiguousarray(np.concatenate(qt_parts, axis=1)),
            "kt8p": np.ascontiguousarray(
                k8.reshape(lc_a, P, TP, 2, P)
                .transpose(4, 0, 2, 3, 1)
                .reshape(P, -1)
            ),
            "kt16p": np.ascontiguousarray(
                k16.reshape(lc_a, P, DC, P).transpose(3, 0, 2, 1).reshape(P, -1)
            ),
            "qm": np.ascontiguousarray(qmk.reshape(jt, P).T),
            "km": np.ascontiguousarray(kmk.reshape(lc_a, P).T),
        }
    per_g = {}
    for g in range(2):
        sl = slice(g * OW, (g + 1) * OW)
        wq8 = (Wq[sl, :].T * WSCALE).astype(E4M3)  # (D, OW)
        wk8 = (Wk[sl, :].T * WSCALE).astype(E4M3)
        per_g[g] = {
            "wq8p": np.ascontiguousarray(
                wq8.reshape(TP, 2, P, HL, P).transpose(2, 0, 3, 1, 4).reshape(P, -1)
            ),
            "wk8p": np.ascontiguousarray(
                wk8.reshape(TP, 2, P, OW).transpose(2, 0, 1, 3).reshape(P, -1)
            ),
            "wv": np.ascontiguousarray(Wv[sl, :].T.astype(np.float16)),
        }

    in_maps = []
    for c in range(NCORES):
        n, g = c // 2, c % 2
        in_maps.append({**per_n[n], **per_g[g]})

    res = bu.run_bass_kernel_spmd(
        nc, in_maps, core_ids=list(range(NCORES)), trace=TRACE
    )
    LAST_EXEC_TIME_NS = res.exec_time_ns

    full = np.zeros((N, L, D), dtype=np.float32)
    for c in range(NCORES):
        n, g = c // 2, c % 2
        o = res.results[c]["out"].astype(np.float32)
        full[n, kept_q[n], g * OW : (g + 1) * OW] = o[: len(kept_q[n])]
    return full


# revision 23
# speedup vs baseline: 1.0088x; 1.0066x over previous
"""Trainium2 Bass kernel for nn_MultiHeadAttention_89429809037632.

Linear attention (softplus feature map) with padding masks:
    q = query @ Wq.T ; k = key @ Wk.T ; v = key @ Wv.T   (per-head split)
    pq = softplus(q) ; pk = softplus(k) * keep(key_mask)
    kv = pk^T v (per head, plus a fused ones-column giving sum(pk))
    out = (pq @ kv) / (pq @ sum(pk)) * keep(query_mask)

Sharding across 8 NeuronCores: data-parallel over N=4 batches x
tensor-parallel over 2 head-groups (8 heads x 128 dims = 1024 output
dims each). Host transposes/packs activations and weights so the
contraction dim (D) lands on the SBUF partition axis; each core runs an
identical SPMD program on its shard, outputs are concatenated on host.

Precision plan (measured ~6e-3 end rel err vs the 2e-2 gate):
  Q and K projections run as fp8(e4m3) Double-Row matmuls (2 fp8
  weights/cell -> 256-deep contraction per pass; measured 109 ns per
  256-col matmul vs 216 ns fp16 = 1.98x). Weights are pre-scaled by 32
  on host to dodge e4m3 subnormals; the 1/32 descale rides the softplus
  EXP activation scale. The V projection stays fp16: v-path
  quantization noise does not average out in the attention sum (the
  output is ~60x smaller than v elements) and fp8 there costs 4e-2
  end-to-end error. softplus is computed directly as ln(1+e^k)
  (|k| <= ~7 so e^k is safe in fp32), 2 ACT ops per 512 cols.

Padding-mask compaction: masked keys contribute exactly zero (pk is
zeroed) and masked query rows are zeroed in the output, so the host
gathers kept rows and the device processes only ceil(kept/128)*128
tokens (~3712 of 4096 at the 10% mask rate). Padded slots carry zero
inputs and zero keep-mask, which reproduces the reference exactly.
The program is compiled for the observed kept counts (cached per
shape), so any mask still yields a correct (if slower) kernel.

Per-core program (Tile framework):
  Phase A: per 128-key chunk: K-proj via 32 DR matmuls (stationary =
    key^T d-pair, moving = packed Wk pairs), V-proj via 32 fp16
    matmuls; softplus+mask -> pk (fp16), V copied into a [v | 1]
    block layout, then 8 per-head matmuls accumulate kv_aug
    (128x129 per head) in PSUM across all key chunks.
  Phase B: per query chunk (<=512 tokens) x head: Q-proj via DR
    matmuls, softplus -> pq, one matmul per 128-query subchunk against
    kv_aug gives [num | den]; epilogue computes num * (keep/den) on
    DVE into a per-chunk staging tile shipped as one fp16 DMA.
  Matmul emission is software-pipelined (kv/num matmuls trail their
  producer chunk by one step). The first Q-projection borrows a
  phase-A PSUM tile and is emitted before the last kv matmuls, so the
  phase boundary costs almost no PE idle. Host packs kt/qt tiles so
  each chunk needs one DMA per tensor, and the weight preload is
  split/interleaved across three DGE queues so chunk 0 is not paced by
  a monolithic 8 MB transfer.
"""

import json
import os
import sys
import types

import numpy as np

for _p in ("/opt/trn_rl_repo",):
    if _p not in sys.path and os.path.isdir(_p):
        sys.path.insert(0, _p)

# ``run_bass_kernel_spmd(trace=True)`` imports antenv.axon_hooks, which not
# every image ships. Provide a stub so the import never crashes (returning
# None simply disables NTFF tracing).
try:
    import antenv.axon_hooks  # noqa: F401
except Exception:
    try:
        import antenv

        _m = types.ModuleType("antenv.axon_hooks")
        _HOOK = [None]

        def _get_hook():
            if _HOOK[0] is None:
                try:
                    from trn_agent_boot.trn_boot import _ntff_profile_via_ctypes

                    _HOOK[0] = _ntff_profile_via_ctypes("/opt/axon/libaxon_pjrt.so")
                except Exception:
                    _HOOK[0] = None
            return _HOOK[0]

        _m.get_axon_ntff_profile_hook = _get_hook
        _m.set_axon_ntff_profile_hook = lambda h: _HOOK.__setitem__(0, h)
        sys.modules["antenv.axon_hooks"] = _m
        antenv.axon_hooks = _m
    except Exception:
        pass

import ml_dtypes

import concourse.bass as bass
import concourse.bass_utils as bu
import concourse.mybir as mybir
import concourse.tile as tile

# ---------------------------------------------------------------------------
# Shim 1: this container's walrus accepts only ONE sync-wait per instruction
# ("Too many sync wait commands"); Tile attaches several. Rewrite the BIR
# JSON so excess waits ride on same-engine NoOps immediately before the
# instruction (engine streams are in-order, so this is equivalent).
# Shim 2: upload_artifacts wants a cloud bucket; keep artifacts local.
# ---------------------------------------------------------------------------
_MAX_WAITS = 1


def _split_multi_waits(bir_bytes: bytes) -> bytes:
    d = json.loads(bir_bytes)
    ctr = 0
    changed = False
    for fn in d.get("functions", []):
        for bb in fn.get("blocks", []):
            out = []
            for inst in bb.get("instructions", []):
                si = inst.get("sync_info")
                waits = (si or {}).get("on_wait") or []
                if len(waits) > _MAX_WAITS:
                    changed = True
                    idx = 0
                    while len(waits) - idx > _MAX_WAITS:
                        chunk = waits[idx : idx + _MAX_WAITS]
                        idx += _MAX_WAITS
                        ctr += 1
                        nop = {
                            "engine": inst["engine"],
                            "ins": [],
                            "outs": [],
                            "name": f"I-wsplit-{ctr}",
                            "opcode": "NoOp",
                            "sync_info": {"on_update": [], "on_wait": chunk},
                        }
                        if "debug" in inst:
                            nop["debug"] = inst["debug"]
                        out.append(nop)
                    si["on_wait"] = waits[idx:]
                out.append(inst)
            bb["instructions"] = out
    return json.dumps(d).encode() if changed else bir_bytes


if not getattr(bass.Bass, "_wait_split_shim", False):
    _orig_to_json = bass.Bass.to_json_bytes

    def _to_json_bytes(self) -> bytes:
        return _split_multi_waits(_orig_to_json(self))

    bass.Bass.to_json_bytes = _to_json_bytes
    bass.Bass._wait_split_shim = True
    bu.upload_artifacts = lambda tmpdir: tmpdir

# ---------------------------------------------------------------------------
# Problem shapes (hardcoded per contract)
# ---------------------------------------------------------------------------
N, L, D = 4, 4096, 2048  # batches, seq len (q and k), model dim
H, P = 16, 128  # heads, head dim
NCORES = 8
HL = H // 2  # heads per core (head-group of 8)
OW = HL * P  # per-core projected width (1024)
DC = D // P  # 16 contraction chunks
TP = D // 256  # 8 d-pair steps for DoubleRow
WSCALE = 32.0  # host premultiplier on fp8 weights (descaled in softplus EXP)

F32 = mybir.dt.float32
F16 = mybir.dt.float16
FP8 = mybir.dt.float8e4
DR = mybir.MatmulPerfMode.DoubleRow
E4M3 = ml_dtypes.float8_e4m3
EXP = mybir.ActivationFunctionType.Exp
LN = mybir.ActivationFunctionType.Ln
MUL = mybir.AluOpType.mult

# kv_aug per-head column offsets inside the 3-bank PSUM accumulator:
# 3 heads per 2 KiB bank (129 fp32 columns each, none crossing a bank edge).
_KV_BASE = [(h // 3) * 512 + (h % 3) * 129 for h in range(HL)]

TRACE = False  # set True (e.g. from test.py) to capture NTFF profile
LAST_EXEC_TIME_NS = None

_CACHED_NC = {}


def _build_nc(lc_a: int, jt: int) -> bass.Bass:
    """lc_a = number of 128-key chunks; jt = number of 128-query blocks."""
    from contextlib import ExitStack

    lq = jt * P  # padded query count
    lc_b = (jt + 3) // 4  # 512-query chunks (last may be partial)

    nc = bass.Bass()
    # host-packed inputs (see kernel() for the exact layouts)
    qt8p = nc.dram_tensor("qt8p", (P, jt * 2048), FP8, kind="ExternalInput")
    kt8p = nc.dram_tensor("kt8p", (P, lc_a * 2048), FP8, kind="ExternalInput")
    kt16p = nc.dram_tensor("kt16p", (P, lc_a * 2048), F16, kind="ExternalInput")
    wq8p = nc.dram_tensor("wq8p", (P, TP * 2048), FP8, kind="ExternalInput")
    wk8p = nc.dram_tensor("wk8p", (P, TP * 2048), FP8, kind="ExternalInput")
    wv = nc.dram_tensor("wv", (D, OW), F16, kind="ExternalInput")
    qm = nc.dram_tensor("qm", (P, jt), F32, kind="ExternalInput")
    km = nc.dram_tensor("km", (P, lc_a), F32, kind="ExternalInput")
    out = nc.dram_tensor("out", (lq, OW), F16, kind="ExternalOutput")

    with tile.TileContext(nc) as tc, ExitStack() as outer:
        misc = outer.enter_context(tc.tile_pool(name="misc", bufs=1))
        wqp = outer.enter_context(tc.tile_pool(name="wqe", bufs=1))
        # SBUF-side phase B pools live in the outer scope; their PSUM
        # counterparts are created only after phase A's pools close
        # (PSUM plan: A = proj 5 + kv 3; B = qp 4 + nm 4).
        qtp = outer.enter_context(tc.tile_pool(name="qt", bufs=2))
        pqp = outer.enter_context(tc.tile_pool(name="pq", bufs=3))
        stp = outer.enter_context(tc.tile_pool(name="st", bufs=3))

        qm_sb = misc.tile([P, jt], F32)
        km_sb = misc.tile([P, lc_a], F32)
        kv_sb = misc.tile([P, HL * 129], F16)
        nc.sync.dma_start(qm_sb[:], qm[:])
        nc.sync.dma_start(km_sb[:], km[:])

        # ------ Phase A: K/V projection + kv accumulation ------------------
        esA = ExitStack()
        wkvp = esA.enter_context(tc.tile_pool(name="wkv", bufs=1))
        ktp = esA.enter_context(tc.tile_pool(name="kt", bufs=3))
        pkp = esA.enter_context(tc.tile_pool(name="pk", bufs=3))
        pps = esA.enter_context(tc.tile_pool(name="projps", bufs=5, space="PSUM"))
        kvps = esA.enter_context(tc.tile_pool(name="kvps", bufs=1, space="PSUM"))
        kv_ps = kvps.tile([P, 1536], F32)

        def load_kt_chunk(c):
            t8 = ktp.tile([P, 2048], FP8, tag="kt8", name="kt8")
            t16 = ktp.tile([P, 2048], F16, tag="kt16", name="kt16")
            nc.sync.dma_start(t8[:], kt8p[:, c * 2048 : (c + 1) * 2048])
            nc.sync.dma_start(t16[:], kt16p[:, c * 2048 : (c + 1) * 2048])
            return t8, t16

        # Chunk 0-2 kt DMAs go first on the sync DGE; weights are split and
        # interleaved across the DGE queues so the opening K matmuls wait
        # on ~2.25 MB and the V matmuls see wv tiles trickling in on two
        # queues rather than queued behind a monolithic 8 MB preload.
        kt_pre = [load_kt_chunk(c) for c in range(min(3, lc_a))]

        wk_sb = wkvp.tile([P, TP * 2048], FP8, name="wk8")
        wv_sb = [
            wkvp.tile([P, OW], F16, tag=f"wv{dc}", name=f"wv{dc}")
            for dc in range(DC)
        ]
        for t in range(TP):
            # wk gates the very first matmul: split it across two DGEs
            eng = nc.gpsimd if t % 2 == 0 else nc.scalar
            eng.dma_start(
                wk_sb[:, t * 2048 : (t + 1) * 2048],
                wk8p[:, t * 2048 : (t + 1) * 2048],
            )
        for dc in range(DC):
            # all of wv rides the scalar DGE behind wk's odd half; the sync
            # DGE carries only kt chunks so the K stream never starves.
            nc.scalar.dma_start(wv_sb[dc][:], wv[dc * P : (dc + 1) * P, :])
        wq_sb = wqp.tile([P, TP * 2048], FP8, name="wq8")

        kv_bank_start = {}

        def emit_kv_mms(c, pk_sb, va_sb):
            for h in range(HL):
                bank_first = h % 3 == 0
                mm = nc.tensor.matmul(
                    kv_ps[:, _KV_BASE[h] : _KV_BASE[h] + 129],
                    pk_sb[:, h * P : (h + 1) * P],
                    va_sb[:, h * 129 : (h + 1) * 129],
                    start=(c == 0 and bank_first),
                    stop=(c == lc_a - 1),
                    skip_group_check=True,
                )
                if c == 0:
                    # start=True clears has_written for the whole PSUM bank;
                    # siblings must come after their bank's clear.
                    if bank_first:
                        kv_bank_start[h // 3] = mm
                    else:
                        tile.add_dep_helper(
                            mm.ins,
                            kv_bank_start[h // 3].ins,
                            reason="kv bank has_written clear order",
                        )

        # kv matmuls for chunk c are emitted after chunk c+1's projection
        # matmuls: their pk operand is only ready ~3us after chunk c's last
        # projection, so this keeps PE fed meanwhile.
        pending = None
        last_k_mm = None
        for c in range(lc_a):
            kt8, kt16 = kt_pre[c] if c < 3 else load_kt_chunk(c)
            if c == min(10, lc_a - 1) and last_k_mm is not None:
                # wq is only needed at the phase boundary; gating its DMA on
                # a chunk-9 matmul keeps the 2 MB transfer out of the
                # HBM-bound startup burst.
                dma = nc.gpsimd.dma_start(wq_sb[:], wq8p[:])
                tile.add_dep_helper(
                    dma.ins, last_k_mm.ins, reason="defer wq past startup"
                )
            elif c == min(10, lc_a - 1):
                nc.gpsimd.dma_start(wq_sb[:], wq8p[:])
            kp0 = pps.tile([P, 512], F32, tag="proj", name="kp0")
            kp1 = pps.tile([P, 512], F32, tag="proj", name="kp1")
            vp0 = pps.tile([P, 512], F32, tag="proj", name="vp0")
            vp1 = pps.tile([P, 512], F32, tag="proj", name="vp1")
            # K projection: fp8 DoubleRow, 256-deep contraction per step.
            # Each 512-col PSUM bank takes two 256-col matmuls; only the
            # first may carry start=True (start clears the whole bank).
            bank_start = {}
            for t in range(TP):
                stat = kt8[:, t * 256 : (t + 1) * 256].rearrange(
                    "p (i m) -> p i m", i=2
                )
                for b in range(4):
                    kp = kp0 if b < 2 else kp1
                    mov = wk_sb[:, t * 2048 : (t + 1) * 2048].rearrange(
                        "p (i o) -> p i o", i=2
                    )[:, :, b * 256 : (b + 1) * 256]
                    first = b % 2 == 0
                    mm = nc.tensor.matmul(
                        kp[:, (b % 2) * 256 : (b % 2 + 1) * 256],
                        stat,
                        mov,
                        start=(t == 0 and first),
                        stop=(t == TP - 1),
                        perf_mode=DR,
                        skip_group_check=True,
                    )
                    if t == 0:
                        if first:
                            bank_start[b // 2] = mm
                        else:
                            tile.add_dep_helper(
                                mm.ins,
                                bank_start[b // 2].ins,
                                reason="kp bank has_written clear order",
                            )
                    last_k_mm = mm
            # V projection: fp16 full-rate (fp8 noise here does not average
            # out in the attention sum; costs 4e-2 end-to-end error).
            for dc in range(DC):
                lhsT = kt16[:, dc * P : (dc + 1) * P]
                st = dict(start=(dc == 0), stop=(dc == DC - 1))
                nc.tensor.matmul(vp0[:], lhsT, wv_sb[dc][:, 0:512], **st)
                nc.tensor.matmul(vp1[:], lhsT, wv_sb[dc][:, 512:1024], **st)

            if pending is not None:
                emit_kv_mms(*pending)

            # softplus(k) = ln(1 + e^k); kp holds 32k so EXP descales.
            pk_sb = pkp.tile([P, OW], F16, tag="pk", name="pk")
            for half, kp in ((0, kp0), (1, kp1)):
                sa = pkp.tile([P, 512], F32, tag=f"sa{half}", name=f"sa{half}")
                nc.scalar.activation(sa[:], kp[:], EXP, scale=1.0 / WSCALE)
                nc.scalar.activation(
                    pk_sb[:, half * 512 : (half + 1) * 512], sa[:], LN, bias=1.0
                )
            nc.vector.tensor_scalar_mul(pk_sb[:], pk_sb[:], km_sb[:, c : c + 1])

            va_sb = pkp.tile([P, HL * 129], F16, tag="vaug", name="va")
            va3 = va_sb[:].rearrange("p (h x) -> p h x", x=129)
            nc.vector.memset(va3[:, :, 128:129], 1.0)
            nc.vector.tensor_copy(
                va3[:, 0:4, 0:P], vp0[:].rearrange("p (h x) -> p h x", x=P)
            )
            nc.vector.tensor_copy(
                va3[:, 4:8, 0:P], vp1[:].rearrange("p (h x) -> p h x", x=P)
            )
            pending = (c, pk_sb, va_sb)

        # ------ Phase B: Q projection + attention epilogue -----------------
        st_tiles = {}
        nmps = None
        scp = pqp  # sc tiles ride the pq pool

        def emit_num(lc, h, jcnt, pq_sb):
            # results stage into st (partition=l%128, cols j*OW+o); heads
            # 0-3 ship at h==3 so only half the staging waits on the tail.
            if h == 0:
                st_tiles[lc] = stp.tile([P, 4 * OW], F16, tag="st", name="st")
            st = st_tiles[lc]
            for j in range(jcnt):
                nm = nmps.tile([P, 129], F32, tag="nm", name="nm")
                nc.tensor.matmul(
                    nm[:],
                    pq_sb[:, j * P : (j + 1) * P],
                    kv_sb[:, h * 129 : h * 129 + 129],
                    start=True,
                    stop=True,
                )
                sc = scp.tile([P, 1], F32, tag="sc", name="sc")
                nc.vector.reciprocal(sc[:], nm[:, 128:129])
                col = lc * 4 + j
                nc.vector.tensor_scalar(
                    st[:, j * OW + h * P : j * OW + (h + 1) * P],
                    nm[:, 0:P],
                    sc[:, 0:1],
                    qm_sb[:, col : col + 1],
                    MUL,
                    MUL,
                )
            if h == 3 or h == HL - 1:
                half = slice(0, 512) if h == 3 else slice(512, 1024)
                nc.sync.dma_start(
                    out[lc * 512 : lc * 512 + jcnt * P, half].rearrange(
                        "(j p) o -> p j o", p=P
                    ),
                    st[:, 0 : jcnt * OW]
                    .rearrange("p (j o) -> p j o", o=OW)[:, :, half],
                )
                if h == HL - 1:
                    del st_tiles[lc]

        # num matmuls for step (lc,h) are emitted after step (lc,h)+1's
        # projection matmuls (pq is ~1.5us of ACT behind qp). The first
        # Q-projection borrows a phase-A PSUM tile and runs BEFORE the
        # last kv matmuls, hiding the final chunk's softplus drain.
        pendingB = None
        qpps = None
        for lc in range(lc_b):
            jcnt = min(4, jt - lc * 4)
            qt8 = qtp.tile([P, 2048 * jcnt], FP8, tag="qt8", name="qt8")
            nc.gpsimd.dma_start(
                qt8[:], qt8p[:, lc * 8192 : lc * 8192 + 2048 * jcnt]
            )
            nt = jcnt * P  # tokens this chunk
            for h in range(HL):
                first_step = lc == 0 and h == 0
                if first_step:
                    qp = pps.tile([P, 512], F32, tag="proj", name="qp0")
                else:
                    qp = qpps.tile([P, 512], F32, tag="qp", name="qp")
                qp_start = None
                for t in range(TP):
                    stat = wq_sb[
                        :, t * 2048 + h * 256 : t * 2048 + (h + 1) * 256
                    ].rearrange("p (i o) -> p i o", i=2)
                    for b0 in range(0, nt, 256):
                        bn = min(256, nt - b0)
                        mov = qt8[
                            :, t * nt * 2 : (t + 1) * nt * 2
                        ].rearrange("p (i n) -> p i n", i=2)[
                            :, :, b0 : b0 + bn
                        ]
                        mm = nc.tensor.matmul(
                            qp[:, b0 : b0 + bn],
                            stat,
                            mov,
                            start=(t == 0 and b0 == 0),
                            stop=(t == TP - 1),
                            perf_mode=DR,
                            skip_group_check=True,
                        )
                        if t == 0:
                            if b0 == 0:
                                qp_start = mm
                            else:
                                tile.add_dep_helper(
                                    mm.ins,
                                    qp_start.ins,
                                    reason="qp bank has_written clear order",
                                )
                if pendingB is not None:
                    emit_num(*pendingB)
                pq_sb = pqp.tile([P, 512], F16, tag="pq", name="pq")
                sa = pqp.tile([P, 512], F32, tag="sqa", name="sqa")
                nc.scalar.activation(
                    sa[:, 0:nt], qp[:, 0:nt], EXP, scale=1.0 / WSCALE
                )
                nc.scalar.activation(pq_sb[:, 0:nt], sa[:, 0:nt], LN, bias=1.0)
                pendingB = (lc, h, jcnt, pq_sb)
                if first_step:
                    # phase A epilogue rides behind the first Q-projection
                    emit_kv_mms(*pending)
                    for hh in range(HL):
                        nc.vector.tensor_copy(
                            kv_sb[:, hh * 129 : (hh + 1) * 129],
                            kv_ps[:, _KV_BASE[hh] : _KV_BASE[hh] + 129],
                        )
                    esA.close()
                    qpps = outer.enter_context(
                        tc.tile_pool(name="qpps", bufs=4, space="PSUM")
                    )
                    nmps = outer.enter_context(
                        tc.tile_pool(name="nmps", bufs=4, space="PSUM")
                    )
        emit_num(*pendingB)
    return nc


def _get_nc(lc_a: int, jt: int) -> bass.Bass:
    if (lc_a, jt) not in _CACHED_NC:
        _CACHED_NC[(lc_a, jt)] = _build_nc(lc_a, jt)
    return _CACHED_NC[(lc_a, jt)]


def kernel(query, key, Wq, Wk, Wv, query_padding_mask, key_padding_mask):
    global LAST_EXEC_TIME_NS
    query = np.asarray(query, dtype=np.float32)
    key = np.asarray(key, dtype=np.float32)
    Wq = np.asarray(Wq, dtype=np.float32)
    Wk = np.asarray(Wk, dtype=np.float32)
    Wv = np.asarray(Wv, dtype=np.float32)
    qmask = np.asarray(query_padding_mask)
    kmask = np.asarray(key_padding_mask)

    # Compact away masked tokens (exact: masked keys contribute zero via
    # the zeroed keep-mask; masked query rows are zeroed on scatter-back).
    kept_k = [np.flatnonzero(~kmask[n]) for n in range(N)]
    kept_q = [np.flatnonzero(~qmask[n]) for n in range(N)]
    lc_a = max(1, -(-max(len(i) for i in kept_k) // P))
    jt = max(1, -(-max(len(i) for i in kept_q) // P))
    lk, lq = lc_a * P, jt * P

    nc = _get_nc(lc_a, jt)

    # Packed layouts (p is always the SBUF partition index, d = 256t+128i+p):
    #   qt8p[p, lc*8192 + t*(2*nt) + i*nt + n] = fp8(query_c[lc*512+n, d])
    #   kt8p[p, c*2048 + t*256 + i*128 + m]    = fp8(key_c[c*128+m, d])
    #   kt16p[p, c*2048 + dc*128 + m]          = fp16(key_c[c*128+m, dc*128+p])
    #   wk8p[p, t*2048 + i*1024 + o]           = fp8(32*Wk[g*OW+o, d])
    #   wq8p[p, t*2048 + h*256 + i*128 + o]    = fp8(32*Wq[g*OW+h*128+o, d])
    per_n = {}
    for n in range(N):
        kc = np.zeros((lk, D), np.float32)
        kc[: len(kept_k[n])] = key[n][kept_k[n]]
        qc = np.zeros((lq, D), np.float32)
        qc[: len(kept_q[n])] = query[n][kept_q[n]]
        kmk = np.zeros(lk, np.float32)
        kmk[: len(kept_k[n])] = 1.0
        qmk = np.zeros(lq, np.float32)
        qmk[: len(kept_q[n])] = 1.0
        q8 = qc.astype(E4M3)
        k8 = kc.astype(E4M3)
        k16 = kc.astype(np.float16)
        # qt8p: per 512-token chunk (last may be short), layout t-major
        qt_parts = []
        for lc in range((jt + 3) // 4):
            nt = min(512, lq - lc * 512)
            blk = q8[lc * 512 : lc * 512 + nt]  # (nt, D)
            qt_parts.append(
                blk.reshape(nt, TP, 2, P).transpose(3, 1, 2, 0).reshape(P, -1)
            )
        per_n[n] = {
            "qt8p": np.ascontiguousarray(np.concatenate(qt_parts, axis=1)),
            "kt8p": np.ascontiguousarray(
                k8.reshape(lc_a, P, TP, 2, P)
                .transpose(4, 0, 2, 3, 1)
                .reshape(P, -1)
            ),
            "kt16p": np.ascontiguousarray(
                k16.reshape(lc_a, P, DC, P).transpose(3, 0, 2, 1).reshape(P, -1)
            ),
            "qm": np.ascontiguousarray(qmk.reshape(jt, P).T),
            "km": np.ascontiguousarray(kmk.reshape(lc_a, P).T),
        }
    per_g = {}
    for g in range(2):
        sl = slice(g * OW, (g + 1) * OW)
        wq8 = (Wq[sl, :].T * WSCALE).astype(E4M3)  # (D, OW)
        wk8 = (Wk[sl, :].T * WSCALE).astype(E4M3)
        per_g[g] = {
            "wq8p": np.ascontiguousarray(
                wq8.reshape(TP, 2, P, HL, P).transpose(2, 0, 3, 1, 4).reshape(P, -1)
            ),
            "wk8p": np.ascontiguousarray(
                wk8.reshape(TP, 2, P, OW).transpose(2, 0, 1, 3).reshape(P, -1)
            ),
            "wv": np.ascontiguousarray(Wv[sl, :].T.astype(np.float16)),
        }

    in_maps = []
    for c in range(NCORES):
        n, g = c // 2, c % 2
        in_maps.append({**per_n[n], **per_g[g]})

    res = bu.run_bass_kernel_spmd(
        nc, in_maps, core_ids=list(range(NCORES)), trace=TRACE
    )
    LAST_EXEC_TIME_NS = res.exec_time_ns

    full = np.zeros((N, L, D), dtype=np.float32)
    for c in range(NCORES):
        n, g = c // 2, c % 2
        o = res.results[c]["out"].astype(np.float32)
        full[n, kept_q[n], g * OW : (g + 1) * OW] = o[: len(kept_q[n])]
    return full
